# revision 7
# baseline (speedup 1.0000x reference)
"""Trainium2 Bass kernel: GQA causal self-attention block (B=1, T=2048, D=2048,
32 q-heads / 8 kv-heads, head_dim 64) with q/k/v/o projections.

Sharding: head-parallel (tensor parallel) across 8 NeuronCores.
Core c owns q-heads 4c..4c+3 and kv-head c. The host sums the 8 partial
outputs (the tensor-parallel reduction).

v4: bf16 activations/weights (fp32 PSUM accumulation), bf16 partial output.
DMA diet: weights land in SBUF-layout order on the host (few big DMAs, small
first pieces so the first matmul starts early), x streams as [128, 1024]
half-rows in consumption order, the output goes out as one [128, 2048] DMA
per row block. Attention ST+exp units for later q-blocks are pre-emitted
into earlier blocks' ACT slack (a drip queue, like the o_proj drip), so the
PE never waits on the scalar engine's exp in the tail q-blocks.
"""

import os
import numpy as np

T = 2048
D = 2048
HQ, HKV = 32, 8
DH = 64
NCORES = 8
PAIRS = 2                 # 2 head-pairs per core (4 q heads)
NCH = D // 128            # 16 contraction chunks for projections
NTQ = 4                   # t-quarters in projection phase
TQW = T // NTQ            # 512
NQB = 4                   # q blocks of 512
QBW = 512
NKB = T // 128            # 16 k blocks of 128

_NC = None
LAST_RESULT = None


def build_nc(dump=False):
    import concourse.tile as tile
    from concourse import bacc, mybir
    from concourse.masks import make_identity, make_upper_triangular

    f32 = mybir.dt.float32
    f32r = mybir.dt.float32r
    bf16 = mybir.dt.bfloat16
    Exp = mybir.ActivationFunctionType.Exp

    nc = bacc.Bacc("TRN2", target_bir_lowering=False, debug=False,
                   num_devices=NCORES)

    xt = nc.dram_tensor("xt", [D, T], bf16, kind="ExternalInput").ap()
    # host-prepared SBUF-layout weights: one partition-row per line
    qpt = nc.dram_tensor("qpt", [128, NCH * 4 * DH], bf16,
                         kind="ExternalInput").ap()
    kvpt = nc.dram_tensor("kvpt", [128, NCH * 2 * DH], bf16,
                          kind="ExternalInput").ap()
    opj = nc.dram_tensor("opj", [128, 2 * D], bf16, kind="ExternalInput").ap()
    out = nc.dram_tensor("out", [T, D], bf16, kind="ExternalOutput").ap()

    from contextlib import ExitStack
    with tile.TileContext(nc) as tc, ExitStack() as ctx:
        consts = ctx.enter_context(tc.tile_pool(name="consts", bufs=1))
        wpool = ctx.enter_context(tc.tile_pool(name="weights", bufs=1))
        qtp = ctx.enter_context(tc.tile_pool(name="qt", bufs=1))
        ktp = ctx.enter_context(tc.tile_pool(name="kt", bufs=1))
        vpool = ctx.enter_context(tc.tile_pool(name="v", bufs=1))
        xpool = ctx.enter_context(tc.tile_pool(name="xhalf", bufs=NCH))
        epool = ctx.enter_context(tc.tile_pool(name="exps", bufs=28))
        cpool = ctx.enter_context(tc.tile_pool(name="ctxsb", bufs=1))
        spool = ctx.enter_context(tc.tile_pool(name="stage", bufs=2))
        opool = ctx.enter_context(tc.tile_pool(name="outsb", bufs=3))
        rpool = ctx.enter_context(tc.tile_pool(name="recip", bufs=2))
        # attention ST psum at stack bottom (banks 0-3) so phase B's ST/exp
        # can overlap phase A (whose psum lives in banks 4-7)
        stp = ctx.enter_context(tc.tile_pool(name="st_ps", bufs=2,
                                             space="PSUM"))

        # constants (built in f32 -- memset/affine_select write f32 --
        # then converted via tensor_copy)
        identf = consts.tile([128, 128], f32, tag="identf")
        make_identity(nc, identf)
        ident = consts.tile([128, 128], bf16, tag="ident")
        nc.vector.tensor_copy(out=ident, in_=identf)
        # mask[i, j] = 1.0 if i <= j else 0  (keep k_row <= q_col)
        maskf = consts.tile([128, 128], f32, tag="maskf")
        make_upper_triangular(nc, maskf, val=1.0, diag=True)
        mask = consts.tile([128, 128], bf16, tag="mask")
        nc.vector.tensor_copy(out=mask, in_=maskf)
        onescf = consts.tile([128, NKB], f32, tag="onescf")
        nc.vector.memset(onescf, 1.0)
        onesc = consts.tile([128, NKB], bf16, tag="onesc")
        nc.vector.tensor_copy(out=onesc, in_=onescf)
        onesrf = consts.tile([65, 64], f32, tag="onesrf")
        nc.vector.memset(onesrf, 1.0)
        onesr = consts.tile([65, 64], f32r, tag="onesr")
        nc.vector.tensor_copy(out=onesr, in_=onesrf)

        # PE warm-up: dummy matmuls during the DMA lead-in keep the HAM
        # activity monitor busy so the first real matmuls run at 2.4 GHz
        with tc.tile_pool(name="warm_ps", bufs=1, space="PSUM") as wps:
            wtile = wps.tile([128, 128], f32, tag="warm")
            for _ in range(8):
                nc.tensor.matmul(wtile, lhsT=identf, rhs=identf,
                                 start=True, stop=True)

        # weights -> SBUF in a few big DMAs (host already in SBUF layout)
        qpt_sb = wpool.tile([128, NCH, 4 * DH], bf16, tag="qpt")
        kvw_sb = wpool.tile([128, NCH, 2 * DH], bf16, tag="kvw")
        opj_sb = wpool.tile([128, 2, D], bf16, tag="opj")
        qpt_r = qpt.rearrange("p (c n) -> p c n", c=NCH)
        kvw_r = kvpt.rearrange("p (c n) -> p c n", c=NCH)
        opj_r2 = opj.rearrange("p (r j) -> p r j", r=2)

        # activation storage
        # qt_sb[p]: rows 0-63 = head 2p (Q^T), rows 64-127 = head 2p+1
        qt_sb = [qtp.tile([128, T], bf16, tag=f"qt{p}", name=f"qt{p}")
                 for p in range(PAIRS)]
        # kv_sb: rows 0-63 = V^T, rows 64-127 = K^T  (kvpt = [v | k])
        kv_sb = ktp.tile([128, T], bf16, tag="kv")
        # K^T copy on partitions 0-63 (for the tile_position (0,0) ST matmul)
        kt_a = ktp.tile([64, T], bf16, tag="kta")
        # V natural [k, dh] per k-block, with a ones column at dh (denominator)
        v_sb = vpool.tile([128, NKB, DH + 1], bf16, tag="vsb")
        nc.vector.tensor_copy(out=v_sb[:, :, DH], in_=onesc)
        # per-pair stacked normalized ctx^T: rows 0-63 head 2p, 64-127 head 2p+1
        ctx_sb = [cpool.tile([128, T], bf16, tag=f"ctx{p}", name=f"ctxsb{p}")
                  for p in range(PAIRS)]

        # x streamed as [128, 1024] half-rows: half A (cols 0:1024) covers
        # tq 0-1, half B covers tq 2-3; B reuses A's pool slot per chunk
        xhalf = {}

        # ---------------- helpers for interleaved emission ----------------
        pending_ex = {}

        def emit_stexp(qb, p, kb):
            q0 = QBW * qb
            kb_off = max(0, 128 * kb - q0)
            st = stp.tile([128, 1024], f32, tag="st", name="st")
            nc.tensor.matmul(
                st[:, kb_off:512],
                lhsT=kt_a[:, 128 * kb:128 * kb + 128],
                rhs=qt_sb[p][0:64, q0 + kb_off:q0 + QBW],
                start=True, stop=True, tile_position=(0, 0))
            nc.tensor.matmul(
                st[:, 512 + kb_off:1024],
                lhsT=kv_sb[64:128, 128 * kb:128 * kb + 128],
                rhs=qt_sb[p][64:128, q0 + kb_off:q0 + QBW],
                start=True, stop=True, tile_position=(64, 0))
            ex = epool.tile([128, 1024], bf16, tag="ex", name="ex")
            if kb_off == 0:
                nc.scalar.activation(out=ex, in_=st, func=Exp)
            else:
                # one 3D-AP exp covering both heads' live columns
                st3 = st.rearrange("p (h q) -> p h q", h=2)
                ex3 = ex.rearrange("p (h q) -> p h q", h=2)
                nc.scalar.activation(
                    out=ex3[:, :, kb_off:512],
                    in_=st3[:, :, kb_off:512], func=Exp)
            if 128 * kb >= q0:  # diagonal block: causal mask (both heads
                # in one 3D-AP multiply; mask broadcast along the head dim)
                ex3m = ex.rearrange("p (h q) -> p h q", h=2)
                nc.vector.tensor_mul(
                    ex3m[:, :, kb_off:kb_off + 128],
                    ex3m[:, :, kb_off:kb_off + 128],
                    mask.rearrange("p (h w) -> p h w", h=1).to_broadcast(
                        [128, 2, 128]))
            return ex

        # ---------------- Phase A: projections (t-quarters) ----------------
        with tc.tile_pool(name="pa_ps", bufs=1, space="PSUM") as pa:
            for tq in range(NTQ):
                t0 = TQW * tq
                qt_ps = [pa.tile([128, TQW], f32, tag=f"qtps{m}",
                                 name=f"qtps{m}") for m in range(2)]
                kv_ps = pa.tile([128, TQW], f32, tag="kvps")
                for ci in range(NCH):
                    if tq == 0:
                        # interleave the weight loads (4+2+2 pieces, small
                        # first so the first matmul starts early) on the ACT
                        # HWDGE ring; x half-rows stream on SP in
                        # consumption order
                        if ci < 4:
                            nc.scalar.dma_start(
                                out=qpt_sb[:, 4 * ci:4 * ci + 4],
                                in_=qpt_r[:, 4 * ci:4 * ci + 4])
                        if ci < 2:
                            nc.scalar.dma_start(
                                out=kvw_sb[:, 8 * ci:8 * ci + 8],
                                in_=kvw_r[:, 8 * ci:8 * ci + 8])
                        elif ci < 4:
                            nc.scalar.dma_start(out=opj_sb[:, ci - 2],
                                                in_=opj_r2[:, ci - 2])
                    if tq == 0:
                        xh = xpool.tile([128, 2 * TQW], bf16, tag="xh",
                                        name=f"xh{ci}_0")
                        nc.sync.dma_start(
                            out=xh, in_=xt[128 * ci:128 * ci + 128, 0:1024])
                        xhalf[(ci, 0)] = xh
                    xc = xhalf[(ci, tq // 2)][:, (t0 % 1024):(t0 % 1024) + TQW]
                    for m in range(2):
                        nc.tensor.matmul(
                            qt_ps[m],
                            lhsT=qpt_sb[:, ci, 128 * m:128 * m + 128],
                            rhs=xc,
                            start=(ci == 0), stop=(ci == NCH - 1))
                    nc.tensor.matmul(
                        kv_ps, lhsT=kvw_sb[:, ci, :], rhs=xc,
                        start=(ci == 0), stop=(ci == NCH - 1))
                    if tq == 1:
                        # issue half B (for tq 2-3) one quarter ahead of use;
                        # emitted after this step's matmuls so the pool's WAR
                        # tracking covers half A's last reader
                        xh = xpool.tile([128, 2 * TQW], bf16, tag="xh",
                                        name=f"xh{ci}_1")
                        nc.sync.dma_start(
                            out=xh, in_=xt[128 * ci:128 * ci + 128, 1024:2048])
                        xhalf[(ci, 1)] = xh
                for m in range(2):
                    nc.any.tensor_copy(out=qt_sb[m][:, t0:t0 + TQW],
                                       in_=qt_ps[m])
                nc.any.tensor_copy(out=kv_sb[:, t0:t0 + TQW], in_=kv_ps)
                # K^T duplicate for this quarter (cross-partition SBUF DMA)
                nc.sync.dma_start(out=kt_a[:, t0:t0 + TQW],
                                  in_=kv_sb[64:128, t0:t0 + TQW])
                # V natural via PE transpose of this quarter's V^T blocks
                for c in range(4 * tq, 4 * tq + 4):
                    tp = pa.tile([128, 64], bf16, tag="vtr", name="vtr")
                    nc.tensor.transpose(
                        tp, in_=kv_sb[0:64, 128 * c:128 * c + 128],
                        identity=ident[0:64, 0:64])
                    nc.any.tensor_copy(out=v_sb[:, c, 0:DH], in_=tp)
                # pre-emit attention ST/exp (runs in A's PE/ACT gaps)
                if tq == 0:
                    for p in range(PAIRS):
                        for kb in range(4):
                            pending_ex[(0, p, kb)] = emit_stexp(0, p, kb)
                elif tq == 1:
                    for kb in range(4):
                        pending_ex[(1, 0, kb)] = emit_stexp(1, 0, kb)
                elif tq == 2:
                    for kb in range(4):
                        pending_ex[(1, 1, kb)] = emit_stexp(1, 1, kb)

        # ---------------- Phase B (attention) + C (o_proj) ----------------
        with tc.tile_pool(name="ctx_ps", bufs=1, space="PSUM") as cxp, \
             tc.tile_pool(name="oc_ps", bufs=2, space="PSUM") as ocp:
            # C-unit state: emit o_proj tiles of the previous qb in drips
            cstate = {"units": [], "osb": None, "tt": -1}

            def emit_cunit():
                if not cstate["units"]:
                    return
                tt, jn = cstate["units"].pop(0)
                if cstate["tt"] != tt:
                    cstate["osb"] = opool.tile([128, D], bf16, tag="osb",
                                               name="osb")
                    cstate["tt"] = tt
                osb = cstate["osb"]
                oc = ocp.tile([128, 512], f32, tag="oc", name="oc")
                for p in range(PAIRS):
                    nc.tensor.matmul(
                        oc,
                        lhsT=ctx_sb[p][:, 128 * tt:128 * tt + 128],
                        rhs=opj_sb[:, p, 512 * jn:512 * jn + 512],
                        start=(p == 0), stop=(p == PAIRS - 1))
                nc.vector.tensor_copy(
                    out=osb[:, 512 * jn:512 * jn + 512], in_=oc)
                if jn == 3:
                    nc.sync.dma_start(
                        out=out[128 * tt:128 * tt + 128, :], in_=osb)

            # ST+exp prefetch queue for later q-blocks: dripped into earlier
            # blocks' kb-steps so the exp (ACT) runs during their slack and
            # the tail q-blocks' AV never waits on the scalar engine
            prefetch = [(q2, p2, k2) for q2 in (2, 3) for p2 in range(PAIRS)
                        for k2 in range(4 * q2 + 4)]
            pfi = {"i": 0}

            def drip_prefetch(cur_qb, cur_p):
                i = pfi["i"]
                if i < len(prefetch) and prefetch[i][0] > cur_qb:
                    q2, p2, k2 = prefetch[i]
                    pending_ex[(q2, p2, k2)] = emit_stexp(q2, p2, k2)
                    pfi["i"] = i + 1

            for qb in range(NQB):
                q0 = QBW * qb
                nkb = 4 * qb + 4
                for p in range(PAIRS):
                    ctx = cxp.tile([DH + 1, 1024], f32, tag="ctx")
                    for kb in range(nkb):
                        ex = pending_ex.pop((qb, p, kb), None)
                        if ex is None:
                            # consuming an un-prefetched unit: emit inline and
                            # keep the prefetch cursor in lockstep
                            if (pfi["i"] < len(prefetch)
                                    and prefetch[pfi["i"]] == (qb, p, kb)):
                                pfi["i"] += 1
                            ex = emit_stexp(qb, p, kb)
                        # ctx^T (+ denominator row 64) accumulation; on
                        # diagonal blocks only cols >= kb_off are live
                        n0 = max(0, 128 * kb - q0)
                        for h in range(2):
                            o = 512 * h
                            nc.tensor.matmul(
                                ctx[:, o + n0:o + 512],
                                lhsT=v_sb[:, kb, :],
                                rhs=ex[:, o + n0:o + 512],
                                start=(kb == 0), stop=(kb == nkb - 1))
                        emit_cunit()
                        drip_prefetch(qb, p)
                    # denominator rows (both heads) -> SBUF (f32r) in one copy
                    densr = rpool.tile([65, 1024], f32r, tag="densr")
                    nc.vector.tensor_copy(
                        out=densr[64:65, :], in_=ctx[64:65, :])
                    for h in range(2):
                        o = 512 * h
                        # replicate down 64 partitions with a K=1 matmul,
                        # then reciprocal
                        repl_ps = ocp.tile([64, 512], f32, tag="oc",
                                           name="replps")
                        nc.tensor.matmul(
                            repl_ps, lhsT=onesr[64:65, 0:64],
                            rhs=densr[64:65, o:o + 512],
                            start=True, stop=True, tile_position=(64, 0))
                        repl = rpool.tile([64, 512], f32, tag="repl")
                        nc.vector.reciprocal(out=repl, in_=repl_ps)
                        if h == 0:
                            nc.vector.tensor_mul(
                                ctx_sb[p][0:64, q0:q0 + QBW],
                                ctx[0:64, o:o + 512], repl)
                        else:
                            stg = spool.tile([64, 512], bf16, tag="stg")
                            nc.vector.tensor_mul(
                                stg, ctx[0:64, o:o + 512], repl)
                            nc.sync.dma_start(
                                out=ctx_sb[p][64:128, q0:q0 + QBW], in_=stg)
                # queue this qb's o_proj tiles; drain leftovers of qb-1 now
                while cstate["units"]:
                    emit_cunit()
                cstate["units"] = [(tt, jn) for tt in range(4 * qb, 4 * qb + 4)
                                   for jn in range(4)]
            while cstate["units"]:
                emit_cunit()

    nc.compile()
    return nc


def _get_nc():
    global _NC
    if _NC is None:
        _NC = build_nc()
    return _NC


def make_in_maps(x, q_proj, k_proj, v_proj, o_proj):
    import ml_dtypes
    bf = ml_dtypes.bfloat16
    x = np.asarray(x, np.float32).reshape(T, D)
    q_proj = np.asarray(q_proj, np.float32)
    k_proj = np.asarray(k_proj, np.float32)
    v_proj = np.asarray(v_proj, np.float32)
    o_proj = np.asarray(o_proj, np.float32)

    xt = np.ascontiguousarray(x.T).astype(bf)  # [D, T]
    scale = 1.0 / np.sqrt(np.float32(DH))

    def sbuf_chunks(w):  # [D, n] -> [128, NCH * n], chunk ci on cols
        n = w.shape[1]
        return np.ascontiguousarray(
            w.reshape(NCH, 128, n).transpose(1, 0, 2).reshape(128, NCH * n))

    maps = []
    for c in range(NCORES):
        qs = slice(4 * DH * c, 4 * DH * (c + 1))     # 256 q rows
        ks = slice(DH * c, DH * (c + 1))             # 64 kv rows
        qptT = (q_proj[qs, :] * scale).T             # [D, 256]
        kvT = np.concatenate([v_proj[ks, :], k_proj[ks, :]], axis=0).T
        opjc = o_proj[qs, :]                         # [256, D]
        # opj_sb[p, r, :] = o_proj[128 r + p]  (r-major within partition)
        opj_host = opjc.reshape(2, 128, D).transpose(1, 0, 2).reshape(128, 2 * D)
        m = {
            "xt": xt,
            "qpt": sbuf_chunks(qptT).astype(bf),
            "kvpt": sbuf_chunks(kvT).astype(bf),
            "opj": np.ascontiguousarray(opj_host).astype(bf),
        }
        maps.append(m)
    return maps


def kernel(**inputs):
    global LAST_RESULT
    from concourse.bass_utils import run_bass_kernel_spmd
    nc = _get_nc()
    maps = make_in_maps(inputs["x"], inputs["q_proj"], inputs["k_proj"],
                        inputs["v_proj"], inputs["o_proj"])
    res = run_bass_kernel_spmd(
        nc, maps, list(range(NCORES)),
        trace=bool(int(os.environ.get("BASS_KERNEL_TRACE", "0"))))
    LAST_RESULT = res
    acc = np.zeros((T, D), np.float64)
    for c in range(NCORES):
        acc += res.results[c]["out"].astype(np.float64)
    return acc.astype(np.float32).reshape(1, T, D)
